# revision 20
# baseline (speedup 1.0000x reference)
"""Trainium2 Bass kernel for nn_ParticleDecoder (retrieval_knn).

Pipeline per NeuronCore (data-parallel over batch, 4 batches/core):
  1. negD = -dist^2 via ONE augmented bf16 matmul (3-way bf16 splits of
     coords & norms -> exact products, fp32 PSUM accumulate).
  2. top-16 (need 13) per query row via DVE max8/max_index/match_replace
     (ties resolved by lowest index, matching jax.lax.top_k).
  3. Neighbor coord gather via GPSIMD indirect_copy from a component-per-
     partition candidate table; index lists and the gathered flatT layout
     are staged through DRAM scratch to reach otherwise inexpressible
     wrapped/transposed layouts with plain affine DMAs.
  4. MLP heads (base + MADE conditioner): layer 1 contracts K=36 in fp32
     PE mode (near-exact); deeper layers run bf16.  Center subtraction,
     +center and biases are folded into matmuls / activation bias APs.
     Output transposed back via PE transposes.
"""

import sys

sys.path.insert(0, "/opt/trn_rl_repo")

import numpy as np
import ml_dtypes

import concourse.bass as bass
import concourse.bacc as bacc
import concourse.mybir as mybir
import concourse.tile as tile
from concourse.bass_utils import run_bass_kernel_spmd

F32 = mybir.dt.float32
BF16 = mybir.dt.bfloat16
U16 = mybir.dt.uint16

B_PER_CORE = 4
N = 1024
NCORES = 8
H = 128
KNN = 12

# product term order (i,j) of the 3-way bf16 splits; must match host prep
ORDER9 = [(0, 0), (0, 1), (1, 0), (0, 2), (1, 1), (2, 0), (1, 2), (2, 1), (2, 2)]

_CACHE: dict = {}
TRACE = False
LAST_RESULTS = None


def _split3(x):
    """Exact 3-way bf16 split: x == s0+s1+s2 (fp32)."""
    x = np.asarray(x, np.float32)
    s0 = x.astype(ml_dtypes.bfloat16).astype(np.float32)
    r1 = (x - s0).astype(np.float32)
    s1 = r1.astype(ml_dtypes.bfloat16).astype(np.float32)
    s2 = (r1 - s1).astype(np.float32)
    return s0, s1, s2


def _made_masks():
    SHELL, DIM, P = 8, 3, 2
    deg_in = np.repeat(np.arange(1, SHELL + 1), DIM)
    deg_h = (np.arange(H) % (SHELL - 1)) + 1
    m2 = (deg_h[None, :] >= deg_h[:, None]).astype(np.float32)
    deg_out = np.repeat(deg_in, P)
    mo = (deg_out[None, :] > deg_h[:, None]).astype(np.float32)
    return m2, mo


def _build_bass():
    nc = bacc.Bacc("TRN2", target_bir_lowering=False)

    # ---------- I/O ----------
    aug_q = nc.dram_tensor("aug_q", [B_PER_CORE, 33, N], BF16, kind="ExternalInput")
    aug_c = nc.dram_tensor("aug_c", [B_PER_CORE, 33, N], BF16, kind="ExternalInput")
    ctr = nc.dram_tensor("ctr", [B_PER_CORE, 3, N], F32, kind="ExternalInput")
    gtab = nc.dram_tensor("gtab", [B_PER_CORE, 128, N], F32, kind="ExternalInput")
    w1 = nc.dram_tensor("w1", [36, H], F32, kind="ExternalInput")
    cw = nc.dram_tensor("cw", [36, H], F32, kind="ExternalInput")
    w1c = nc.dram_tensor("w1c", [3, H], F32, kind="ExternalInput")
    cwc = nc.dram_tensor("cwc", [3, H], F32, kind="ExternalInput")
    w2 = nc.dram_tensor("w2", [H, H], BF16, kind="ExternalInput")
    w3 = nc.dram_tensor("w3", [H, H], BF16, kind="ExternalInput")
    mw2m = nc.dram_tensor("mw2m", [H, H], BF16, kind="ExternalInput")
    wbp = nc.dram_tensor("wbp", [H, 64], BF16, kind="ExternalInput")
    mwo = nc.dram_tensor("mwo", [H, 64], BF16, kind="ExternalInput")
    c48 = nc.dram_tensor("c48", [3, 64], F32, kind="ExternalInput")
    ident = nc.dram_tensor("ident", [64, 64], F32, kind="ExternalInput")
    b1 = nc.dram_tensor("b1", [H, 1], F32, kind="ExternalInput")
    mb1 = nc.dram_tensor("mb1", [H, 1], F32, kind="ExternalInput")
    b2 = nc.dram_tensor("b2", [H, 1], F32, kind="ExternalInput")
    mb2 = nc.dram_tensor("mb2", [H, 1], F32, kind="ExternalInput")
    b3 = nc.dram_tensor("b3", [H, 1], F32, kind="ExternalInput")
    bpred = nc.dram_tensor("bpred", [24, 1], F32, kind="ExternalInput")
    bstd = nc.dram_tensor("bstd", [24, 1], F32, kind="ExternalInput")
    out_d = nc.dram_tensor("out", [B_PER_CORE, N, 48], F32, kind="ExternalOutput")

    with tile.TileContext(nc) as tc:
        with (
            tc.tile_pool(name="wpool", bufs=1) as wp,
            tc.tile_pool(name="cpool", bufs=2) as cp,
            tc.tile_pool(name="dist", bufs=3) as dp,
            tc.tile_pool(name="gpool", bufs=2) as gp,
            tc.tile_pool(name="hpool", bufs=2) as hp,
            tc.tile_pool(name="small", bufs=6) as sp,
            tc.tile_pool(name="pd", bufs=2, space="PSUM") as pd_pool,
            tc.tile_pool(name="ph", bufs=2, space="PSUM") as ph_pool,
            tc.tile_pool(name="po", bufs=1, space="PSUM") as po_pool,
            tc.tile_pool(name="pt", bufs=1, space="PSUM") as pt_pool,
            tc.tile_pool(name="dram", bufs=1, space="DRAM") as dram_pool,
        ):
            # ---------- load constants ----------
            def load_const(src, shape, dtype=F32):
                t = wp.tile(shape, dtype, tag=src.name)
                nc.sync.dma_start(t[:], src.ap())
                return t

            w1_t = load_const(w1, [36, H])
            cw_t = load_const(cw, [36, H])
            w1c_t = load_const(w1c, [3, H])
            cwc_t = load_const(cwc, [3, H])
            w2_t = load_const(w2, [H, H], BF16)
            w3_t = load_const(w3, [H, H], BF16)
            mw2m_t = load_const(mw2m, [H, H], BF16)
            wbp_t = load_const(wbp, [H, 64], BF16)
            mwo_t = load_const(mwo, [H, 64], BF16)
            c48_t = load_const(c48, [3, 64])
            id_t = load_const(ident, [64, 64])
            b1_t = load_const(b1, [H, 1])
            mb1_t = load_const(mb1, [H, 1])
            b2_t = load_const(b2, [H, 1])
            mb2_t = load_const(mb2, [H, 1])
            b3_t = load_const(b3, [H, 1])
            b48_t = wp.tile([64, 1], F32, tag="b48")
            nc.sync.dma_start(b48_t[0:24, :], bpred.ap())
            nc.sync.dma_start(b48_t[32:56, :], bstd.ap())

            scratch = dram_pool.tile([B_PER_CORE, 2, 512, 16], U16)
            scratch2 = dram_pool.tile([B_PER_CORE, 2, 36, 512], F32)

            for b in range(B_PER_CORE):
                ctr_t = cp.tile([3, N], F32, tag="ctr")
                nc.sync.dma_start(ctr_t[:], ctr.ap()[b])
                augq_t = cp.tile([33, N], BF16, tag="augq")
                nc.sync.dma_start(augq_t[:], aug_q.ap()[b])
                augc_t = cp.tile([33, N], BF16, tag="augc")
                nc.sync.dma_start(augc_t[:], aug_c.ap()[b])
                # candidate table: partition 16k+d holds component d
                gtab_t = cp.tile([128, N], F32, tag="gtab")
                nc.sync.dma_start(gtab_t[:], gtab.ap()[b])

                # ---------- top-k over 8 query tiles ----------
                for t in range(8):
                    pd = pd_pool.tile([128, N], F32, tag="pd")
                    lhs = augq_t[:, 128 * t : 128 * (t + 1)]
                    nc.tensor.matmul(
                        pd[:, 0:512], lhs, augc_t[:, 0:512], start=True, stop=True
                    )
                    nc.tensor.matmul(
                        pd[:, 512:1024], lhs, augc_t[:, 512:1024], start=True, stop=True
                    )
                    negd = dp.tile([128, N], F32, tag="negd")
                    nc.scalar.copy(negd[:], pd[:])

                    maxv = sp.tile([128, 16], F32, tag="maxv")
                    idx16 = sp.tile([128, 16], U16, tag="idx16")
                    nc.vector.max(out=maxv[:, 0:8], in_=negd[:])
                    nc.vector.max_index(
                        out=idx16[:, 0:8], in_max=maxv[:, 0:8], in_values=negd[:]
                    )
                    nc.vector.match_replace(
                        out=negd[:],
                        in_to_replace=maxv[:, 0:8],
                        in_values=negd[:],
                        imm_value=-1e30,
                    )
                    nc.vector.max(out=maxv[:, 8:16], in_=negd[:])
                    nc.vector.max_index(
                        out=idx16[:, 8:16], in_max=maxv[:, 8:16], in_values=negd[:]
                    )
                    # store contiguous [128, 16] rows for this tile
                    q0 = 128 * (t % 4)
                    nc.scalar.dma_start(
                        scratch[b, t // 4, q0 : q0 + 128, :], idx16[:]
                    )

                # ---------- gather + MLP over 2 groups of 512 queries ----------
                for g in range(2):
                    wrap = sp.tile([128, 64], U16, tag="wrap")
                    scr_v = scratch[b, g].rearrange(
                        "(k qh pl) r -> k qh pl r", k=8, qh=4
                    )
                    for qh in range(4):
                        nc.sync.dma_start(
                            wrap[:, 16 * qh : 16 * qh + 16], scr_v[:, qh, :, :]
                        )

                    gout = gp.tile([128, N], F32, tag="gout")
                    nc.gpsimd.indirect_copy(
                        gout[:], gtab_t[:], wrap[:],
                        i_know_ap_gather_is_preferred=True,
                    )
                    # dump gathered comps to DRAM in flatT order, reload
                    gout_v = gout[d0 : d0 + 113 : 16, :] if False else None
                    scr2_v = scratch2[b, g].rearrange(
                        "p (k qh pl) -> p k qh pl", k=8, qh=4
                    )
                    for d in range(3):
                        srcv = gout[d : d + 113 : 16, :].rearrange(
                            "p (qh r pl) -> p qh r pl", qh=4, r=16
                        )
                        for qh in range(4):
                            srcg = srcv[:, qh, 1 : 1 + KNN, :]
                            dstg = scr2_v[d : d + 34 : 3, :, qh, :].rearrange(
                                "j k pl -> k j pl"
                            )
                            nc.sync.dma_start(dstg, srcg)
                    flatT = gp.tile([36, 512], F32, tag="flatT")
                    nc.sync.dma_start(flatT[:], scratch2[b, g])

                    ctr_s = ctr_t[:, 512 * g : 512 * (g + 1)]

                    ph1 = ph_pool.tile([H, 512], F32, tag="ph")
                    ph1m = ph_pool.tile([H, 512], F32, tag="ph")
                    nc.tensor.matmul(ph1[:], w1c_t[:], ctr_s, start=True, stop=False)
                    nc.tensor.matmul(ph1[:], w1_t[:], flatT[:], start=False, stop=True)
                    nc.tensor.matmul(ph1m[:], cwc_t[:], ctr_s, start=True, stop=False)
                    nc.tensor.matmul(
                        ph1m[:], cw_t[:], flatT[:], start=False, stop=True
                    )

                    h1 = hp.tile([H, 512], BF16, tag="h")
                    nc.scalar.activation(
                        h1[:], ph1[:], mybir.ActivationFunctionType.Relu, bias=b1_t[:]
                    )
                    h1m = hp.tile([H, 512], BF16, tag="h")
                    nc.scalar.activation(
                        h1m[:], ph1m[:], mybir.ActivationFunctionType.Tanh,
                        bias=mb1_t[:],
                    )

                    ph2 = ph_pool.tile([H, 512], F32, tag="ph")
                    nc.tensor.matmul(ph2[:], w2_t[:], h1[:], start=True, stop=True)
                    h2 = hp.tile([H, 512], BF16, tag="h")
                    nc.scalar.activation(
                        h2[:], ph2[:], mybir.ActivationFunctionType.Relu, bias=b2_t[:]
                    )

                    ph2m = ph_pool.tile([H, 512], F32, tag="ph")
                    nc.tensor.matmul(ph2m[:], mw2m_t[:], h1m[:], start=True, stop=True)
                    h2m = hp.tile([H, 512], BF16, tag="h")
                    nc.scalar.activation(
                        h2m[:], ph2m[:], mybir.ActivationFunctionType.Tanh,
                        bias=mb2_t[:],
                    )

                    ph3 = ph_pool.tile([H, 512], F32, tag="ph")
                    nc.tensor.matmul(ph3[:], w3_t[:], h2[:], start=True, stop=True)
                    h3 = hp.tile([H, 512], BF16, tag="h")
                    nc.scalar.activation(
                        h3[:], ph3[:], mybir.ActivationFunctionType.Relu, bias=b3_t[:]
                    )

                    po = po_pool.tile([64, 512], F32, tag="po")
                    nc.tensor.matmul(po[:], wbp_t[:], h3[:], start=True, stop=False)
                    nc.tensor.matmul(po[:], mwo_t[:], h2m[:], start=False, stop=False)
                    nc.tensor.matmul(po[:], c48_t[:], ctr_s, start=False, stop=True)

                    outs = hp.tile([64, 512], F32, tag="outs")
                    nc.scalar.activation(
                        outs[0:24, :], po[0:24, :],
                        mybir.ActivationFunctionType.Identity, bias=b48_t[0:24, :],
                    )
                    nc.scalar.activation(
                        outs[32:56, :], po[32:56, :],
                        mybir.ActivationFunctionType.Exp, bias=b48_t[32:56, :],
                        scale=0.5,
                    )

                    for tt in range(4):
                        pt = pt_pool.tile([128, 64], F32, tag="pt")
                        nc.tensor.transpose(
                            pt[:], outs[:, 128 * tt : 128 * (tt + 1)], id_t[:]
                        )
                        sT = sp.tile([128, 64], F32, tag="sT")
                        nc.scalar.copy(sT[:], pt[:])
                        q0 = 512 * g + 128 * tt
                        dview = out_d.ap()[b, q0 : q0 + 128, :].rearrange(
                            "q (j h) -> q h j", h=2
                        )
                        nc.gpsimd.dma_start(dview[:, 0, :], sT[:, 0:24])
                        nc.gpsimd.dma_start(dview[:, 1, :], sT[:, 32:56])

    nc.compile()
    return nc


def _prep_host(inputs):
    """Host-side prep of per-core in_maps (numpy only)."""
    coords = np.asarray(inputs["coords"], np.float32)  # [32, 1024, 3]

    m2, mo = _made_masks()
    w_h1 = np.asarray(inputs["w_h1"], np.float32)
    cw_w = np.asarray(inputs["cw"], np.float32)
    perm = np.concatenate([np.arange(0, 48, 2), np.arange(1, 48, 2)])

    tobf = lambda a: np.asarray(a, np.float32).astype(ml_dtypes.bfloat16)

    wbp_p = np.zeros((H, 64), np.float32)
    wbp_p[:, 0:24] = np.asarray(inputs["w_bp"], np.float32)[:, perm[:24]]
    wbp_p[:, 32:56] = np.asarray(inputs["w_bp"], np.float32)[:, perm[24:]]
    mwo_full = np.asarray(inputs["mwo"], np.float32) * mo
    mwo_p = np.zeros((H, 64), np.float32)
    mwo_p[:, 0:24] = mwo_full[:, perm[:24]]
    mwo_p[:, 32:56] = mwo_full[:, perm[24:]]
    bias48 = (
        np.asarray(inputs["b_bp"], np.float32) + np.asarray(inputs["mbo"], np.float32)
    )[perm]
    bpred_v = bias48[:24].reshape(24, 1).copy()
    bstd_v = (0.5 * bias48[24:]).reshape(24, 1).copy()

    w1c_v = -w_h1.reshape(12, 3, H).sum(0, dtype=np.float64).astype(np.float32)
    cwc_v = -cw_w.reshape(12, 3, H).sum(0, dtype=np.float64).astype(np.float32)
    mw2m_v = (np.asarray(inputs["mw2"], np.float32) * m2).copy()

    c48_v = np.zeros((3, 64), np.float32)
    for m in range(24):
        c48_v[m % 3, m] = 1.0
    ident_v = np.eye(64, dtype=np.float32)

    shared = {
        "w1": w_h1,
        "cw": cw_w,
        "w1c": w1c_v,
        "cwc": cwc_v,
        "w2": tobf(inputs["w_h2"]),
        "w3": tobf(inputs["w_h3"]),
        "mw2m": tobf(mw2m_v),
        "wbp": tobf(wbp_p),
        "mwo": tobf(mwo_p),
        "c48": c48_v,
        "ident": ident_v,
        "b1": np.asarray(inputs["b_h1"], np.float32).reshape(H, 1),
        "mb1": np.asarray(inputs["mb1"], np.float32).reshape(H, 1),
        "b2": np.asarray(inputs["b_h2"], np.float32).reshape(H, 1),
        "mb2": np.asarray(inputs["mb2"], np.float32).reshape(H, 1),
        "b3": np.asarray(inputs["b_h3"], np.float32).reshape(H, 1),
        "bpred": bpred_v,
        "bstd": bstd_v,
    }

    in_maps = []
    for core in range(NCORES):
        cs = coords[core * B_PER_CORE : (core + 1) * B_PER_CORE]  # [4,1024,3]
        aug_q = np.zeros((B_PER_CORE, 33, N), np.float32)
        aug_c = np.zeros((B_PER_CORE, 33, N), np.float32)
        ctr_v = np.zeros((B_PER_CORE, 3, N), np.float32)
        gtab_v = np.zeros((B_PER_CORE, 128, N), np.float32)
        for bb in range(B_PER_CORE):
            c = cs[bb]  # [1024, 3]
            x2 = (c * c).astype(np.float32)
            sq = ((x2[:, 0] + x2[:, 1]) + x2[:, 2]).astype(np.float32)
            qs = [_split3(2.0 * c[:, d]) for d in range(3)]
            csd = [_split3(c[:, d]) for d in range(3)]
            nsq = _split3(-sq)
            r = 0
            for d in range(3):
                for (i, j) in ORDER9:
                    aug_q[bb, r] = qs[d][i]
                    aug_c[bb, r] = csd[d][j]
                    r += 1
            for i in range(3):
                aug_q[bb, r] = nsq[i]
                aug_c[bb, r] = 1.0
                r += 1
            for j in range(3):
                aug_q[bb, r] = 1.0
                aug_c[bb, r] = nsq[j]
                r += 1
            ctr_v[bb] = c.T
            for k in range(8):
                gtab_v[bb, 16 * k : 16 * k + 3, :] = c.T
        im = dict(shared)
        im["aug_q"] = aug_q.astype(ml_dtypes.bfloat16)
        im["aug_c"] = aug_c.astype(ml_dtypes.bfloat16)
        im["ctr"] = ctr_v
        im["gtab"] = gtab_v
        in_maps.append(im)
    return in_maps


def kernel(**inputs) -> np.ndarray:
    global LAST_RESULTS
    if "nc" not in _CACHE:
        _CACHE["nc"] = _build_bass()
    nc = _CACHE["nc"]
    in_maps = _prep_host(inputs)
    res = run_bass_kernel_spmd(
        nc, in_maps, core_ids=list(range(NCORES)), trace=TRACE
    )
    LAST_RESULTS = res
    outs = [res.results[c]["out"] for c in range(NCORES)]  # [4, 1024, 48] each
    full = np.concatenate(outs, axis=0)  # [32, 1024, 48]
    return full.reshape(32, N, 8, 3, 2).astype(np.float32)
